# revision 5
# baseline (speedup 1.0000x reference)
"""RNN-T Joiner kernel for 8 Trainium2 NeuronCores.

Reference computation (per batch element n):
    enc = encoder_out[n] @ W_enc.T + b_enc          # (T=200, J=512)
    dec = decoder_out[n] @ W_dec.T + b_dec          # (U=50,  J=512)
    x   = tanh(enc[:,None,:] + dec[None,:,:])       # (T, U, J)
    out = x @ W_out.T + b_out                       # (T, U, V=500)

Sharding: data-parallel over N=8 (one batch element per core).

Device-side dataflow (j/c-major, pre-transposed on host):
    PE:     encT/decT projections, main matmul with W_out stationary and
            x moving -> logits produced v-major: [V(part), TU(free)]
    GPSIMD: S[j,t,u] = encT[j,t] + decT[j,u]  (broadcast add, bf16 out)
    ACT:    XT = tanh(S)  (bf16)
    DVE:    one PSUM->SBUF evacuation + b_out bias instr per 500-tu chunk
            (b_out is per-partition in this orientation)
    DMA:    250KB v-major output stores; host transposes to (T,U,V)
"""

import numpy as np

N, T, U = 8, 200, 50
C = 512   # enc/dec feature dim
J = 512   # joint dim
V = 500   # vocab
TU = T * U
P = 128
KC = J // P          # 4 contraction chunks of 128
VT = 4               # vocab tiles of 125 rows
VR = V // VT         # 125
CH_T = 10            # t's per GEMM chunk
CH = CH_T * U        # 500 tu per GEMM chunk (one PSUM bank per vt)
NCH = T // CH_T      # 20 chunks
XG = 2               # GEMM chunks per x-production chunk
XCH = CH * XG        # 1000 tu per x chunk
NXCH = NCH // XG     # 10

_CACHE = {}


def _build_bass():
    import concourse.bass as bass  # noqa: F401
    import concourse.mybir as mybir
    import concourse.tile as tile
    from concourse import bacc

    bf16 = mybir.dt.bfloat16
    f32 = mybir.dt.float32
    Act = mybir.ActivationFunctionType

    nc = bacc.Bacc("TRN2", target_bir_lowering=False, debug=False, num_devices=N)

    enc_in = nc.dram_tensor("enc_in", [C, T], bf16, kind="ExternalInput").ap()
    dec_in = nc.dram_tensor("dec_in", [C, U], bf16, kind="ExternalInput").ap()
    w_enc = nc.dram_tensor("w_enc", [C, J], bf16, kind="ExternalInput").ap()
    w_dec = nc.dram_tensor("w_dec", [C, J], bf16, kind="ExternalInput").ap()
    w_out = nc.dram_tensor("w_out", [J, V], bf16, kind="ExternalInput").ap()
    biases = nc.dram_tensor("biases", [P, 2 * KC + VT], f32,
                            kind="ExternalInput").ap()
    # logits stored v-major: [V, TU]; host transposes back to (T,U,V)
    logits = nc.dram_tensor("logits_v", [V, TU], bf16,
                            kind="ExternalOutput").ap()
    logits_r = logits.rearrange("(vt p) tu -> p vt tu", p=VR)

    with tile.TileContext(nc) as tc:
        with (
            tc.tile_pool(name="const", bufs=1) as const,
            tc.tile_pool(name="s", bufs=3) as sp,
            tc.tile_pool(name="xt", bufs=3) as xtp,
            tc.tile_pool(name="lout", bufs=2) as lp,
            tc.tile_pool(name="ps", bufs=2, space="PSUM") as psp,
        ):
            # ---- load weights + inputs -------------------------------------
            w_enc_sb = const.tile([P, KC, J], bf16)
            w_dec_sb = const.tile([P, KC, J], bf16)
            w_out_sb = const.tile([P, KC, V], bf16)
            enc_in_sb = const.tile([P, KC, T], bf16)
            dec_in_sb = const.tile([P, KC, U], bf16)
            bias_sb = const.tile([P, 2 * KC + VT], f32)
            b_enc_sb = bias_sb[:, 0:KC]
            b_dec_sb = bias_sb[:, KC:2 * KC]
            b_out_sb = bias_sb[:, 2 * KC:]

            enc_in_r = enc_in.rearrange("(kc p) t -> p kc t", p=P)
            dec_in_r = dec_in.rearrange("(kc p) u -> p kc u", p=P)
            w_enc_r = w_enc.rearrange("(kc p) j -> p kc j", p=P)
            w_dec_r = w_dec.rearrange("(kc p) j -> p kc j", p=P)
            w_out_r = w_out.rearrange("(kc p) v -> p kc v", p=P)

            # critical path first: enc proj needs enc_in + w_enc jc-cols
            nc.sync.dma_start(bias_sb[:], biases)
            nc.sync.dma_start(dec_in_sb[:], dec_in_r)
            nc.sync.dma_start(w_dec_sb[:], w_dec_r)
            nc.scalar.dma_start(enc_in_sb[:], enc_in_r)
            # w_enc in jc-column chunks so projections can start early
            for jc in range(KC):
                nc.scalar.dma_start(
                    w_enc_sb[:, :, jc * P:(jc + 1) * P],
                    w_enc_r[:, :, jc * P:(jc + 1) * P])
            nc.gpsimd.dma_start(w_out_sb[:], w_out_r)

            # ---- input projections, in transposed (j-major) form -----------
            encT = const.tile([P, KC, T], f32)
            decT = const.tile([P, KC, U], f32)
            for jc in range(KC):
                ps = psp.tile([P, VT, 512], f32, tag="ps", name="pse")
                for kc in range(KC):
                    nc.tensor.matmul(
                        ps[:, 0, :T],
                        lhsT=w_enc_sb[:, kc, jc * P:(jc + 1) * P],
                        rhs=enc_in_sb[:, kc, :],
                        start=(kc == 0),
                        stop=(kc == KC - 1),
                    )
                nc.scalar.activation(
                    encT[:, jc, :], ps[:, 0, :T], Act.Identity,
                    bias=b_enc_sb[:, jc:jc + 1],
                )
                ps = psp.tile([P, VT, 512], f32, tag="ps", name="psd")
                for kc in range(KC):
                    nc.tensor.matmul(
                        ps[:, 0, :U],
                        lhsT=w_dec_sb[:, kc, jc * P:(jc + 1) * P],
                        rhs=dec_in_sb[:, kc, :],
                        start=(kc == 0),
                        stop=(kc == KC - 1),
                    )
                nc.scalar.activation(
                    decT[:, jc, :], ps[:, 0, :U], Act.Identity,
                    bias=b_dec_sb[:, jc:jc + 1],
                )

            # ---- steady-state loop over 500-tu chunks ----------------------
            xts = [None] * KC

            def produce_x(xc, sub):
                """Produce x for x-chunk xc (XCH tu), split into `sub` pieces
                per kc so the first GEMM matmul can start early."""
                t0 = xc * XG * CH_T
                nt = XG * CH_T
                step = nt // sub
                for kc in range(KC):
                    s = sp.tile([P, nt, U], bf16, tag=f"s{kc}", name=f"s{kc}")
                    x = xtp.tile([P, nt, U], bf16, tag=f"x{kc}", name=f"x{kc}")
                    xts[kc] = x.rearrange("p t u -> p (t u)")
                    for i in range(sub):
                        lo, hi = i * step, (i + 1) * step
                        nc.gpsimd.tensor_add(
                            s[:, lo:hi, :],
                            encT[:, kc, t0 + lo:t0 + hi, None]
                            .to_broadcast((P, hi - lo, U)),
                            decT[:, kc, None, :].to_broadcast((P, hi - lo, U)),
                        )
                        nc.scalar.activation(
                            x[:, lo:hi, :], s[:, lo:hi, :], Act.Tanh
                        )

            L = None
            for c in range(NCH):
                if c % XG == 0:
                    produce_x(c // XG, 4 if c == 0 else 1)
                    L = lp.tile([P, VT, XG, CH], bf16, tag="L", name="L")
                sl = c % XG
                ps = psp.tile([P, VT, 512], f32, tag="ps", name="psm")
                for vt in range(VT):
                    for kc in range(KC):
                        nc.tensor.matmul(
                            ps[:VR, vt, :CH],
                            lhsT=w_out_sb[:, kc, vt * VR:(vt + 1) * VR],
                            rhs=xts[kc][:, sl * CH:(sl + 1) * CH],
                            start=(kc == 0),
                            stop=(kc == KC - 1),
                        )
                # single bias-add evacuation for all 4 vocab tiles
                nc.vector.tensor_add(
                    L[:VR, :, sl, :],
                    ps[:VR, :, :CH],
                    b_out_sb[:VR, :, None].to_broadcast((VR, VT, CH)),
                )
                if sl == XG - 1:
                    g = c // XG
                    nc.sync.dma_start(
                        logits_r[:, :, g * XCH:(g + 1) * XCH],
                        L[:VR, :, :, :],
                    )

    nc.compile()
    return nc


def _get_bass():
    if "nc" not in _CACHE:
        _CACHE["nc"] = _build_bass()
    return _CACHE["nc"]


def _pack_inputs(inputs):
    import ml_dtypes

    encoder_out = np.ascontiguousarray(
        np.asarray(inputs["encoder_out"], np.float32).transpose(0, 2, 1)
        .astype(ml_dtypes.bfloat16))
    decoder_out = np.ascontiguousarray(
        np.asarray(inputs["decoder_out"], np.float32).transpose(0, 2, 1)
        .astype(ml_dtypes.bfloat16))
    WencT = np.ascontiguousarray(
        np.asarray(inputs["W_enc"], np.float32).T.astype(ml_dtypes.bfloat16))
    WdecT = np.ascontiguousarray(
        np.asarray(inputs["W_dec"], np.float32).T.astype(ml_dtypes.bfloat16))
    WoutT = np.ascontiguousarray(
        np.asarray(inputs["W_out"], np.float32).T.astype(ml_dtypes.bfloat16))
    biases = np.zeros((P, 2 * KC + VT), np.float32)
    biases[:, 0:KC] = np.asarray(inputs["b_enc"], np.float32).reshape(KC, P).T
    biases[:, KC:2 * KC] = (
        np.asarray(inputs["b_dec"], np.float32).reshape(KC, P).T)
    b_out = np.asarray(inputs["b_out"], np.float32)
    for vt in range(VT):
        biases[:VR, 2 * KC + vt] = b_out[vt * VR:(vt + 1) * VR]
    return [
        {
            "enc_in": encoder_out[n],
            "dec_in": decoder_out[n],
            "w_enc": WencT,
            "w_dec": WdecT,
            "w_out": WoutT,
            "biases": biases,
        }
        for n in range(N)
    ]


def run(inputs, trace=False):
    """Run the bass kernel; returns (output array, BassKernelResults)."""
    from concourse.bass_utils import run_bass_kernel_spmd

    nc = _get_bass()
    in_maps = _pack_inputs(inputs)
    res = run_bass_kernel_spmd(nc, in_maps, core_ids=list(range(N)), trace=trace)
    out = np.empty((N, T, U, V), np.float32)
    for n, r in enumerate(res.results):
        lv = np.asarray(r["logits_v"], dtype=np.float32)  # [V, TU]
        out[n] = lv.T.reshape(T, U, V)
    return out, res


def kernel(**inputs):
    out, _ = run(inputs)
    return out


# revision 6
# speedup vs baseline: 1.1263x; 1.1263x over previous
"""RNN-T Joiner kernel for 8 Trainium2 NeuronCores.

Reference computation (per batch element n):
    enc = encoder_out[n] @ W_enc.T + b_enc          # (T=200, J=512)
    dec = decoder_out[n] @ W_dec.T + b_dec          # (U=50,  J=512)
    x   = tanh(enc[:,None,:] + dec[None,:,:])       # (T, U, J)
    out = x @ W_out.T + b_out                       # (T, U, V=500)

Sharding: data-parallel over N=8 (one batch element per core).

Device-side dataflow (j/c-major, pre-transposed on host):
    PE:     encT/decT projections, main matmul with W_out stationary and
            x moving -> logits produced v-major: [V(part), TU(free)]
    GPSIMD: S[j,t,u] = encT[j,t] + decT[j,u]  (broadcast add, bf16 out)
    ACT:    XT = tanh(S)  (bf16)
    DVE:    one PSUM->SBUF evacuation + b_out bias instr per 500-tu chunk
            (b_out is per-partition in this orientation)
    DMA:    250KB v-major output stores; host transposes to (T,U,V)
"""

import numpy as np

N, T, U = 8, 200, 50
C = 512   # enc/dec feature dim
J = 512   # joint dim
V = 500   # vocab
VP = 512  # padded vocab (full 128-row tiles -> 16-wide output DMA)
TU = T * U
P = 128
KC = J // P          # 4 contraction chunks of 128
VT = 4               # vocab tiles of 128 rows (padded)
VR = VP // VT        # 128
CH_T = 10            # t's per GEMM chunk
CH = CH_T * U        # 500 tu per GEMM chunk (one PSUM bank per vt)
NCH = T // CH_T      # 20 chunks
XG = 2               # GEMM chunks per x-production chunk
XCH = CH * XG        # 1000 tu per x chunk
NXCH = NCH // XG     # 10

_CACHE = {}


def _build_bass():
    import concourse.bass as bass  # noqa: F401
    import concourse.mybir as mybir
    import concourse.tile as tile
    from concourse import bacc

    bf16 = mybir.dt.bfloat16
    f32 = mybir.dt.float32
    Act = mybir.ActivationFunctionType

    nc = bacc.Bacc("TRN2", target_bir_lowering=False, debug=False, num_devices=N)

    enc_in = nc.dram_tensor("enc_in", [C, T], bf16, kind="ExternalInput").ap()
    dec_in = nc.dram_tensor("dec_in", [C, U], bf16, kind="ExternalInput").ap()
    w_enc = nc.dram_tensor("w_enc", [C, J], bf16, kind="ExternalInput").ap()
    w_dec = nc.dram_tensor("w_dec", [C, J], bf16, kind="ExternalInput").ap()
    w_out = nc.dram_tensor("w_out", [J, VP], bf16, kind="ExternalInput").ap()
    biases = nc.dram_tensor("biases", [P, 2 * KC + VT], f32,
                            kind="ExternalInput").ap()
    # logits stored v-major: [V, TU]; host transposes back to (T,U,V)
    logits = nc.dram_tensor("logits_v", [VP, TU], bf16,
                            kind="ExternalOutput").ap()
    logits_r = logits.rearrange("(vt p) tu -> p vt tu", p=VR)

    with tile.TileContext(nc) as tc:
        with (
            tc.tile_pool(name="const", bufs=1) as const,
            tc.tile_pool(name="s", bufs=3) as sp,
            tc.tile_pool(name="xt", bufs=3) as xtp,
            tc.tile_pool(name="lout", bufs=2) as lp,
            tc.tile_pool(name="ps", bufs=2, space="PSUM") as psp,
        ):
            # ---- load weights + inputs -------------------------------------
            w_enc_sb = const.tile([P, KC, J], bf16)
            w_dec_sb = const.tile([P, KC, J], bf16)
            w_out_sb = const.tile([P, KC, VP], bf16)
            enc_in_sb = const.tile([P, KC, T], bf16)
            dec_in_sb = const.tile([P, KC, U], bf16)
            bias_sb = const.tile([P, 2 * KC + VT], f32)
            b_enc_sb = bias_sb[:, 0:KC]
            b_dec_sb = bias_sb[:, KC:2 * KC]
            b_out_sb = bias_sb[:, 2 * KC:]

            enc_in_r = enc_in.rearrange("(kc p) t -> p kc t", p=P)
            dec_in_r = dec_in.rearrange("(kc p) u -> p kc u", p=P)
            w_enc_r = w_enc.rearrange("(kc p) j -> p kc j", p=P)
            w_dec_r = w_dec.rearrange("(kc p) j -> p kc j", p=P)
            w_out_r = w_out.rearrange("(kc p) v -> p kc v", p=P)

            # critical path first: enc proj needs enc_in + w_enc jc-cols
            nc.sync.dma_start(bias_sb[:], biases)
            nc.sync.dma_start(dec_in_sb[:], dec_in_r)
            nc.sync.dma_start(w_dec_sb[:], w_dec_r)
            nc.scalar.dma_start(enc_in_sb[:], enc_in_r)
            # w_enc in jc-column chunks so projections can start early
            for jc in range(KC):
                nc.scalar.dma_start(
                    w_enc_sb[:, :, jc * P:(jc + 1) * P],
                    w_enc_r[:, :, jc * P:(jc + 1) * P])
            nc.gpsimd.dma_start(w_out_sb[:], w_out_r)

            # ---- input projections, in transposed (j-major) form -----------
            encT = const.tile([P, KC, T], f32)
            decT = const.tile([P, KC, U], f32)
            for jc in range(KC):
                ps = psp.tile([P, VT, 512], f32, tag="ps", name="pse")
                for kc in range(KC):
                    nc.tensor.matmul(
                        ps[:, 0, :T],
                        lhsT=w_enc_sb[:, kc, jc * P:(jc + 1) * P],
                        rhs=enc_in_sb[:, kc, :],
                        start=(kc == 0),
                        stop=(kc == KC - 1),
                    )
                nc.scalar.activation(
                    encT[:, jc, :], ps[:, 0, :T], Act.Identity,
                    bias=b_enc_sb[:, jc:jc + 1],
                )
                ps = psp.tile([P, VT, 512], f32, tag="ps", name="psd")
                for kc in range(KC):
                    nc.tensor.matmul(
                        ps[:, 0, :U],
                        lhsT=w_dec_sb[:, kc, jc * P:(jc + 1) * P],
                        rhs=dec_in_sb[:, kc, :],
                        start=(kc == 0),
                        stop=(kc == KC - 1),
                    )
                nc.scalar.activation(
                    decT[:, jc, :], ps[:, 0, :U], Act.Identity,
                    bias=b_dec_sb[:, jc:jc + 1],
                )

            # ---- steady-state loop over 500-tu chunks ----------------------
            xts = [None] * KC

            def produce_x(xc, sub):
                """Produce x for x-chunk xc (XCH tu), split into `sub` pieces
                per kc so the first GEMM matmul can start early."""
                t0 = xc * XG * CH_T
                nt = XG * CH_T
                step = nt // sub
                for kc in range(KC):
                    s = sp.tile([P, nt, U], bf16, tag=f"s{kc}", name=f"s{kc}")
                    x = xtp.tile([P, nt, U], bf16, tag=f"x{kc}", name=f"x{kc}")
                    xts[kc] = x.rearrange("p t u -> p (t u)")
                    eng = nc.vector if kc == 3 else nc.gpsimd
                    for i in range(sub):
                        lo, hi = i * step, (i + 1) * step
                        eng.tensor_add(
                            s[:, lo:hi, :],
                            encT[:, kc, t0 + lo:t0 + hi, None]
                            .to_broadcast((P, hi - lo, U)),
                            decT[:, kc, None, :].to_broadcast((P, hi - lo, U)),
                        )
                        nc.scalar.activation(
                            x[:, lo:hi, :], s[:, lo:hi, :], Act.Tanh
                        )

            L = None
            for c in range(NCH):
                if c % XG == 0:
                    produce_x(c // XG, 4 if c == 0 else 1)
                    L = lp.tile([P, VT, XG, CH], bf16, tag="L", name="L")
                sl = c % XG
                ps = psp.tile([P, VT, 512], f32, tag="ps", name="psm")
                for vt in range(VT):
                    for kc in range(KC):
                        nc.tensor.matmul(
                            ps[:VR, vt, :CH],
                            lhsT=w_out_sb[:, kc, vt * VR:(vt + 1) * VR],
                            rhs=xts[kc][:, sl * CH:(sl + 1) * CH],
                            start=(kc == 0),
                            stop=(kc == KC - 1),
                        )
                # single bias-add evacuation for all 4 vocab tiles
                nc.vector.tensor_add(
                    L[:VR, :, sl, :],
                    ps[:VR, :, :CH],
                    b_out_sb[:VR, :, None].to_broadcast((VR, VT, CH)),
                )
                if sl == XG - 1:
                    g = c // XG
                    nc.sync.dma_start(
                        logits_r[:, :, g * XCH:(g + 1) * XCH],
                        L[:VR, :, :, :],
                    )

    nc.compile()
    return nc


def _get_bass():
    if "nc" not in _CACHE:
        _CACHE["nc"] = _build_bass()
    return _CACHE["nc"]


def _pack_inputs(inputs):
    import ml_dtypes

    encoder_out = np.ascontiguousarray(
        np.asarray(inputs["encoder_out"], np.float32).transpose(0, 2, 1)
        .astype(ml_dtypes.bfloat16))
    decoder_out = np.ascontiguousarray(
        np.asarray(inputs["decoder_out"], np.float32).transpose(0, 2, 1)
        .astype(ml_dtypes.bfloat16))
    WencT = np.ascontiguousarray(
        np.asarray(inputs["W_enc"], np.float32).T.astype(ml_dtypes.bfloat16))
    WdecT = np.ascontiguousarray(
        np.asarray(inputs["W_dec"], np.float32).T.astype(ml_dtypes.bfloat16))
    WoutT = np.zeros((J, VP), ml_dtypes.bfloat16)
    WoutT[:, :V] = np.asarray(inputs["W_out"], np.float32).T.astype(
        ml_dtypes.bfloat16)
    biases = np.zeros((P, 2 * KC + VT), np.float32)
    biases[:, 0:KC] = np.asarray(inputs["b_enc"], np.float32).reshape(KC, P).T
    biases[:, KC:2 * KC] = (
        np.asarray(inputs["b_dec"], np.float32).reshape(KC, P).T)
    b_out = np.zeros(VP, np.float32)
    b_out[:V] = np.asarray(inputs["b_out"], np.float32)
    for vt in range(VT):
        biases[:, 2 * KC + vt] = b_out[vt * VR:(vt + 1) * VR]
    return [
        {
            "enc_in": encoder_out[n],
            "dec_in": decoder_out[n],
            "w_enc": WencT,
            "w_dec": WdecT,
            "w_out": WoutT,
            "biases": biases,
        }
        for n in range(N)
    ]


def run(inputs, trace=False):
    """Run the bass kernel; returns (output array, BassKernelResults)."""
    from concourse.bass_utils import run_bass_kernel_spmd

    nc = _get_bass()
    in_maps = _pack_inputs(inputs)
    res = run_bass_kernel_spmd(nc, in_maps, core_ids=list(range(N)), trace=trace)
    out = np.empty((N, T, U, V), np.float32)
    for n, r in enumerate(res.results):
        lv = np.asarray(r["logits_v"], dtype=np.float32)  # [VP, TU]
        out[n] = lv[:V].T.reshape(T, U, V)
    return out, res


def kernel(**inputs):
    out, _ = run(inputs)
    return out
